# revision 53
# baseline (speedup 1.0000x reference)
import numpy as np

STACK, UNITS, D, EPS = 12, 4, 128, 1e-3
T = 16                    # t-blocks per group (partition dim: 4 units x 16 t)
NB = 1024                 # cols per t-block
G_ROWS = T * NB           # 16384 rows per group
GROUPS = 4
NPAIR = STACK // 2        # 6 stage pairs
CORE_ROWS = GROUPS * G_ROWS   # 65536
B_FULL = 500000
N_CORES = 8
B_PAD = CORE_ROWS * N_CORES   # 524288


def _bf16(a):
    import ml_dtypes
    return np.asarray(a, np.float32).astype(ml_dtypes.bfloat16)


def _const_layout():
    """Column offsets of each stationary inside the packed `big` tensor."""
    off = {}
    c = 0
    def add(name, w):
        nonlocal c
        off[name] = c
        c += w
    add("wx", 49)
    add("ident", 128)
    for q in range(1, NPAIR):
        for cc in range(q):
            add(f"a_{q}_{cc}", 128)
    for q in range(NPAIR):
        add(f"p_{q}", 64)
    for cc in range(NPAIR):
        add(f"wd_{cc}", 16)
    add("sd", 16)
    return off, c


def prep_consts(inputs):
    """Host-side packing of all stationaries for the pair-fused layout.

    Data layouts on device:
      cx tile  [128, NB]: row 64k + 12u + i  (k: t-block within pcx tile,
               i = 2q+h), row 64k+48 = x-part of the final logit diff.
      cxF      [128, NPAIR*NB]: partition 64h + 16u + t, block q of NB cols;
               holds scale*(x @ w_i) + bias for stage i = 2q + h.
      y2[cc]   [128, NB]: partition 64h + 16v + t = relu output of stage 2cc+h.
    """
    ws = [np.asarray(inputs[f"w{i}"], np.float32) for i in range(STACK)]
    gamma = np.asarray(inputs["gamma"], np.float32)
    beta = np.asarray(inputs["beta"], np.float32)
    mean = np.asarray(inputs["mean"], np.float32)
    var = np.asarray(inputs["var"], np.float32)
    wf = np.asarray(inputs["wf"], np.float32)
    bf = np.asarray(inputs["bf"], np.float32)

    s = gamma / np.sqrt(var + EPS)
    bsh = beta - mean * s
    wd = wf[:, 0] - wf[:, 1]
    bd = float(bf[0] - bf[1])

    off, width = _const_layout()
    big = np.zeros((128, width), np.float32)
    rng16 = np.arange(16)

    # x-projection weights: col 24h + 6u + q for stage i = 2q + h
    # (order chosen so the cx->cxF scatter balances to 3 AP dims);
    # col 48 = x part of logit diff
    o = off["wx"]
    for i in range(STACK):
        q, h = divmod(i, 2)
        for u in range(UNITS):
            big[:, o + 24 * h + 6 * u + q] = ws[i][4 * i:, u] * s[i, u]
    big[:, o + 48] = wd[4 * STACK:]

    big[:, off["ident"]:off["ident"] + 128] = np.eye(128)

    # chunk stationaries: src pair cc (stages 2cc+jj) -> dest pair q (stages 2q+h)
    for q in range(1, NPAIR):
        for cc in range(q):
            o = off[f"a_{q}_{cc}"]
            for jj in range(2):
                j = 2 * cc + jj
                for h in range(2):
                    i = 2 * q + h
                    for v in range(UNITS):
                        for u in range(UNITS):
                            val = ws[i][4 * (i - 1 - j) + v, u] * s[i, u]
                            big[64 * jj + 16 * v + rng16,
                                o + 64 * h + 16 * u + rng16] = val

    # intra-pair partials: y_{2q} -> stage 2q+1 (dest cols 16u+t, M=64)
    for q in range(NPAIR):
        i = 2 * q + 1
        o = off[f"p_{q}"]
        for v in range(UNITS):
            for u in range(UNITS):
                val = ws[i][v, u] * s[i, u]
                big[16 * v + rng16, o + 16 * u + rng16] = val

    # tail: wd coefficients, src pair cc -> single logit-diff row (cols = t)
    for cc in range(NPAIR):
        o = off[f"wd_{cc}"]
        for jj in range(2):
            j = 2 * cc + jj
            for v in range(UNITS):
                big[64 * jj + 16 * v + rng16, o + rng16] = wd[4 * (STACK - 1 - j) + v]

    # d-inject identity (cxFd row t -> pds row t)
    big[rng16, off["sd"] + rng16] = 1.0

    # per-partition bias folded into the pcx->cx copy: row 64k + 24h + 6u + q
    bias = np.zeros((128, 1), np.float32)
    for k in range(2):
        for u in range(UNITS):
            for i in range(STACK):
                q, h = divmod(i, 2)
                bias[64 * k + 24 * h + 6 * u + q, 0] = bsh[i, u]

    return {"big": _bf16(big), "wx": _bf16(big[:, off["wx"]:off["wx"] + 49]),
            "bias": bias, "bd": bd}


DEBUG = False
MM_LABELS = {}     # instruction name -> semantic label (for trace analysis)
_CUR = [""]


def build_kernel(ctx, tc, outs, ins, *, bd):
    import concourse.mybir as mybir

    nc = tc.nc
    f32 = mybir.dt.float32
    bf16 = mybir.dt.bfloat16
    ACT = mybir.ActivationFunctionType
    ALU = mybir.AluOpType

    xg_ap = ins["x"]          # [128, CORE_ROWS] bf16 (pre-transposed on host)
    out_ap = outs["out"]      # [CORE_ROWS, 2] f32

    _real_mm = nc.tensor.matmul

    def _mm(*a, **kw):
        inst = _real_mm(*a, **kw)
        try:
            MM_LABELS[inst.name] = _CUR[0]
        except Exception:
            pass
        return inst
    nc.tensor.matmul = _mm

    off, width = _const_layout()
    const_pool = ctx.enter_context(tc.tile_pool(name="consts", bufs=1))
    # tiny consts first so the x-projection can start immediately; the big
    # stationary pack loads on the gpsimd queue in the background
    wx_t = const_pool.tile([128, 49], bf16, tag="wx", name="wx_t")
    nc.sync.dma_start(wx_t[:], ins["wx"])
    bias_sb = const_pool.tile([128, 1], f32, tag="bias", name="bias_sb")
    nc.scalar.dma_start(bias_sb[:], ins["bias"])
    big = const_pool.tile([128, width], bf16, tag="big", name="big")
    nc.gpsimd.dma_start(big[:], ins["big"])

    wx = wx_t[:]
    ident = big[:, off["ident"]:off["ident"] + 128]
    a_st = {(q, cc): big[:, off[f"a_{q}_{cc}"]:off[f"a_{q}_{cc}"] + 128]
            for q in range(1, NPAIR) for cc in range(q)}
    p_st = {q: big[0:64, off[f"p_{q}"]:off[f"p_{q}"] + 64] for q in range(NPAIR)}
    wd_st = [big[:, off[f"wd_{cc}"]:off[f"wd_{cc}"] + 16] for cc in range(NPAIR)]
    sd = big[0:16, off["sd"]:off["sd"] + 16]

    xt_pool = ctx.enter_context(tc.tile_pool(name="xt", bufs=12))
    cx_pool = ctx.enter_context(tc.tile_pool(name="cx", bufs=6))
    cxf_pool = ctx.enter_context(tc.tile_pool(name="cxf", bufs=4))
    cxfd_pool = ctx.enter_context(tc.tile_pool(name="cxfd", bufs=4))
    y2_pool = ctx.enter_context(tc.tile_pool(name="y2", bufs=3))
    out_pool = ctx.enter_context(tc.tile_pool(name="outsb", bufs=2))
    # PSUM budget (8 banks): pcx ring 2 x 1 + z ring 2 x 2 + pds 2 x 1
    psum_pool = ctx.enter_context(tc.tile_pool(name="ps", bufs=2, space="PSUM"))
    pd_pool = ctx.enter_context(tc.tile_pool(name="pd", bufs=2, space="PSUM"))

    state = {}

    def start_group(g):
        cxF = cxf_pool.tile([128, NPAIR * NB], bf16, tag="cxF", name="cxF")
        cxFd = cxfd_pool.tile([16, NB], bf16, tag="cxFd", name="cxFd")
        y2 = [y2_pool.tile([128, NB], bf16, tag=f"y2c{cc}", name=f"y2c{cc}")
              for cc in range(NPAIR)]
        state[g] = (cxF, cxFd, y2)

    def relu_full(dst, src, flip):
        """Full-width relu on one act engine (fewer per-op overheads)."""
        if not flip:
            nc.scalar.activation(dst, src, ACT.Relu)
        else:
            nc.vector.tensor_scalar(dst, src, 0.0, None, ALU.max)

    def emit_cx_dma(g, j, eng=None):
        xt = xt_pool.tile([128, 2 * NB], bf16, tag="xt", name="xt")
        c0 = g * G_ROWS + j * 2 * NB
        (eng or nc.sync).dma_start(xt[:], xg_ap[:, c0:c0 + 2 * NB])
        return xt

    def emit_cx_compute(g, j, xt, pcx_tag="ps"):
        cxF, cxFd, y2 = state[g]
        _CUR[0] = f"cx{g}.{j}"
        # NB+64 pad: keeps the scatter's q-stride (one partition) from being
        # dim-merged with the 1024-element inner run into a bogus contiguous
        # 6144-element descriptor that would read past the partition.
        cxsb_full = cx_pool.tile([128, NB + 64], bf16, tag="cx")
        cxsb = cxsb_full[:, 0:NB]
        if pcx_tag == "z":
            # prologue-only: borrow a 2-bank z-ring slab for extra depth
            pcx = psum_pool.tile([128, NB], f32, tag="z", name="pcxz")
            for tb in range(2):
                for hh in range(2):
                    nc.tensor.matmul(
                        pcx[64 * tb:64 * tb + 49, 512 * hh:512 * hh + 512],
                        wx, xt[:, NB * tb + 512 * hh:NB * tb + 512 * hh + 512],
                        start=True, stop=True)
            if (j + g) % 2 == 0:
                nc.scalar.activation(cxsb[:], pcx[:], ACT.Identity,
                                     bias=bias_sb[:, 0:1])
            else:
                nc.vector.tensor_scalar(cxsb[:], pcx[:], bias_sb[:, 0:1],
                                        None, ALU.add)
        else:
            for hh in range(2):
                pcx = psum_pool.tile([128, 512], f32, tag="ps", name="pcx")
                for tb in range(2):
                    nc.tensor.matmul(
                        pcx[64 * tb:64 * tb + 49, :],
                        wx, xt[:, NB * tb + 512 * hh:NB * tb + 512 * hh + 512],
                        start=True, stop=True)
                sl = slice(512 * hh, 512 * hh + 512)
                if (j + g + hh) % 2 == 0:
                    nc.scalar.activation(cxsb[:, sl], pcx[:], ACT.Identity,
                                         bias=bias_sb[:, 0:1])
                else:
                    nc.vector.tensor_scalar(cxsb[:, sl], pcx[:], bias_sb[:, 0:1],
                                            None, ALU.add)
        # scatter rows 64k + 24h + 6u + q -> cxF[16*(4h+u) + t], block q.
        # src must stay 2D (the balancer splits 48 -> 8x6); an explicit 3D
        # src AP silently drops the middle dim in SW-DGE descriptor gen.
        for k in range(2):
            dst = cxF[:].rearrange(
                "(p s) (q n) -> p q s n", p=8, s=T, q=NPAIR)[:, :, 2 * j + k]
            eng = nc.sync if k == 0 else nc.gpsimd
            eng.dma_start(dst, cxsb[64 * k:64 * k + 48])
        dsrc = cxsb[:].rearrange("(k c) n -> k c n", k=2)[:, 48]
        nc.gpsimd.dma_start(cxFd[2 * j:2 * j + 2, :], dsrc)

    xt_pending = {}   # g -> list of prefetched xt tiles

    def prefetch_xt(g, upto, split=False):
        lst = xt_pending.setdefault(g, [])
        while len(lst) < min(upto, 8):
            eng = nc.scalar if (split and len(lst) % 2 == 1) else nc.sync
            lst.append(emit_cx_dma(g, len(lst), eng))

    def emit_pair(g, q, midfill=None, prefill=None):
        cxF, cxFd, y2 = state[g]
        z = psum_pool.tile([128, NB], f32, tag="z", name=f"z{q}", bufs=2)
        csl = slice(q * NB, (q + 1) * NB)
        _CUR[0] = f"id{g}.{q}"
        if q == 0:
            # stage 0 is pure relu(cx): read cxF directly, PSUM only for stage 1
            for hh in range(2):
                nc.tensor.matmul(z[64:128, 512 * hh:512 * hh + 512],
                                 ident[:, 64:128],
                                 cxF[:, q * NB + 512 * hh:q * NB + 512 * hh + 512],
                                 start=True, stop=False)
            relu_full(y2[0][0:64, :], cxF[0:64, csl], flip=g % 2 == 1)
        else:
            for hh in range(2):
                nc.tensor.matmul(z[:, 512 * hh:512 * hh + 512], ident,
                                 cxF[:, q * NB + 512 * hh:q * NB + 512 * hh + 512],
                                 start=True, stop=False)
            for cc in range(q):
                if cc == q - 1 and prefill is not None:
                    # filler covers the relu#2 latency of the previous pair
                    prefill()
                _CUR[0] = f"ch{g}.{q}.{cc}"
                for hh in range(2):
                    nc.tensor.matmul(z[:, 512 * hh:512 * hh + 512], a_st[(q, cc)],
                                     y2[cc][:, 512 * hh:512 * hh + 512],
                                     start=False, stop=False,
                                     skip_group_check=True)
            relu_full(y2[q][0:64, :], z[0:64, :], flip=(g + q) % 2 == 1)
        # filler matmuls here cover the relu#1 latency the partial waits on
        if midfill is not None:
            midfill()
        _CUR[0] = f"pa{g}.{q}"
        for hh in range(2):
            nc.tensor.matmul(z[64:128, 512 * hh:512 * hh + 512], p_st[q],
                             y2[q][0:64, 512 * hh:512 * hh + 512],
                             start=False, stop=True, skip_group_check=True)
        relu_full(y2[q][64:128, :], z[64:128, :], flip=(g + q) % 2 == 0)

    tail_state = {}

    def emit_tail_start(g):
        cxF, cxFd, y2 = state[g]
        _CUR[0] = f"tl{g}.s"
        pds = [pd_pool.tile([16, 512], f32, tag="pd", name=f"pds{hh}")
               for hh in range(2)]
        tail_state[g] = pds
        for hh in range(2):
            nc.tensor.matmul(pds[hh][:], sd, cxFd[:, 512 * hh:512 * hh + 512],
                             start=True, stop=False)

    def emit_tail_chunk(g, cc):
        cxF, cxFd, y2 = state[g]
        _CUR[0] = f"tl{g}.c{cc}"
        pds = tail_state[g]
        for hh in range(2):
            nc.tensor.matmul(pds[hh][:], wd_st[cc],
                             y2[cc][:, 512 * hh:512 * hh + 512],
                             start=False, stop=(cc == NPAIR - 1),
                             skip_group_check=True)

    def emit_tail_out(g):
        _CUR[0] = f"tl{g}.o"
        pds = tail_state[g]
        outsb = out_pool.tile([16, 2 * NB], f32, tag="outsb", name="outsb")
        for hh in range(2):
            o2 = outsb[:, 1024 * hh:1024 * hh + 1024].rearrange(
                "p (n two) -> p n two", two=2)
            nc.scalar.activation(o2[:, :, 0], pds[hh][:], ACT.Sigmoid,
                                 bias=float(bd))
            if g < GROUPS - 1:
                # p1 = 1 - p0 on gpsimd (SBUF-only, off the act engines); the
                # last group avoids gpsimd so its SW-DGE drain overlaps compute
                nc.gpsimd.tensor_scalar(o2[:, :, 1], o2[:, :, 0], -1.0, 1.0,
                                        ALU.mult, ALU.add)
            else:
                nc.scalar.activation(o2[:, :, 1], pds[hh][:], ACT.Sigmoid,
                                     bias=float(-bd), scale=-1.0)
        og = out_ap[g * G_ROWS:(g + 1) * G_ROWS, :].rearrange(
            "(t n) two -> t (n two)", t=T)
        nc.scalar.dma_start(og, outsb[:])

    def emit_debug(g):
        cxF, cxFd, y2 = state[g]
        nc.sync.dma_start(outs["dbg_cxf"], cxF[:])
        nc.sync.dma_start(outs["dbg_cxfd"], cxFd[:])
        for cc in range(NPAIR):
            nc.sync.dma_start(outs["dbg_y2"][:, cc * NB:(cc + 1) * NB], y2[cc][:])

    # Software pipeline: group g's pair recurrence interleaves group g+1's
    # x-projection tiles; the previous group's tail is woven in after pair 0.
    start_group(0)
    prefetch_xt(0, 8, split=True)
    prefetch_xt(1, 3, split=True)
    for j in range(8):
        emit_cx_compute(0, j, xt_pending[0][j], pcx_tag=("z" if j % 2 else "ps"))
    pending_tail = None
    for g in range(GROUPS):
        if g + 1 < GROUPS:
            start_group(g + 1)
            prefetch_xt(g + 1, 3)
        emitted = 0

        def fill(upto):
            nonlocal emitted
            if g + 1 >= GROUPS:
                return
            while emitted < min(upto, 8):
                emit_cx_compute(g + 1, emitted, xt_pending[g + 1][emitted])
                emitted += 1
                prefetch_xt(g + 1, emitted + 3)

        PRE = [0, 2, 4, 5, 7, 8]
        MID = [1, 3, 4, 6, 7, 8]

        def mid(q):
            if q == 0:
                fill(2)   # boundary filler: covers the prev group's last relu
            if q == 0 and pending_tail is not None:
                emit_tail_chunk(pending_tail, 5)
                emit_tail_out(pending_tail)
            fill(MID[q])

        for q in range(NPAIR):
            emit_pair(g, q, midfill=lambda q=q: mid(q),
                      prefill=lambda q=q: fill(PRE[q]))
            if q == 0 and pending_tail is not None:
                pending_tail = None
            if q == 1:
                emit_tail_start(g)
            if q >= 1:
                emit_tail_chunk(g, q - 1)
            if q == 4 and g + 2 < GROUPS:
                prefetch_xt(g + 2, 2)   # boundary fills' data, issued early
        pending_tail = g
        if DEBUG and g == 0:
            emit_debug(0)
    emit_tail_chunk(pending_tail, 5)
    emit_tail_out(pending_tail)


# ---------------------------------------------------------------------------
# Self-contained entry point: kernel(**inputs) -> [500000, 2] float32
# ---------------------------------------------------------------------------

import sys as _sys
if '/opt/trn_rl_repo' not in _sys.path:
    _sys.path.insert(0, '/opt/trn_rl_repo')

_CACHE = {}


def _build_nc(bd):
    from contextlib import ExitStack
    import concourse.mybir as mybir
    from concourse import bacc
    import concourse.tile as tile

    _, width = _const_layout()
    nc = bacc.Bacc("TRN2", target_bir_lowering=False, debug=False,
                   num_devices=N_CORES)
    ins = {}
    ins["x"] = nc.dram_tensor("x", [128, CORE_ROWS], mybir.dt.bfloat16,
                              kind="ExternalInput").ap()
    ins["big"] = nc.dram_tensor("big", [128, width], mybir.dt.bfloat16,
                                kind="ExternalInput").ap()
    ins["wx"] = nc.dram_tensor("wx", [128, 49], mybir.dt.bfloat16,
                               kind="ExternalInput").ap()
    ins["bias"] = nc.dram_tensor("bias", [128, 1], mybir.dt.float32,
                                 kind="ExternalInput").ap()
    outs = {"out": nc.dram_tensor("out", [CORE_ROWS, 2], mybir.dt.float32,
                                  kind="ExternalOutput").ap()}
    if DEBUG:
        outs["dbg_cxf"] = nc.dram_tensor(
            "dbg_cxf", [128, NPAIR * NB], mybir.dt.bfloat16,
            kind="ExternalOutput").ap()
        outs["dbg_cxfd"] = nc.dram_tensor(
            "dbg_cxfd", [16, NB], mybir.dt.bfloat16, kind="ExternalOutput").ap()
        outs["dbg_y2"] = nc.dram_tensor(
            "dbg_y2", [128, NPAIR * NB], mybir.dt.bfloat16,
            kind="ExternalOutput").ap()
    with tile.TileContext(nc) as tc:
        with ExitStack() as ctx:
            build_kernel(ctx, tc, outs, ins, bd=bd)
    nc.compile()
    return nc


def make_in_maps(inputs):
    import ml_dtypes

    consts = prep_consts(inputs)
    bd = consts.pop("bd")
    x = np.asarray(inputs["x"], dtype=np.float32)
    assert x.shape == (B_FULL, D)
    xb = np.zeros((B_PAD, D), ml_dtypes.bfloat16)
    xb[:B_FULL] = x.astype(ml_dtypes.bfloat16)

    in_maps = []
    for c in range(N_CORES):
        xt = np.ascontiguousarray(xb[c * CORE_ROWS:(c + 1) * CORE_ROWS].T)
        m = {"x": xt}
        m.update(consts)
        in_maps.append(m)
    return in_maps, bd


def kernel(**inputs):
    from concourse.bass_utils import run_bass_kernel_spmd

    in_maps, bd = make_in_maps(inputs)
    if "nc" not in _CACHE:
        _CACHE["nc"] = _build_nc(bd)
    nc = _CACHE["nc"]
    res = run_bass_kernel_spmd(nc, in_maps, core_ids=list(range(N_CORES)))
    out = np.concatenate([res.results[c]["out"] for c in range(N_CORES)], axis=0)
    return out[:B_FULL]


# revision 54
# speedup vs baseline: 1.0505x; 1.0505x over previous
import numpy as np

STACK, UNITS, D, EPS = 12, 4, 128, 1e-3
T = 16                    # t-blocks per group (partition dim: 4 units x 16 t)
NB = 1024                 # cols per t-block
G_ROWS = T * NB           # 16384 rows per group
GROUPS = 4
NPAIR = STACK // 2        # 6 stage pairs
CORE_ROWS = GROUPS * G_ROWS   # 65536
B_FULL = 500000
N_CORES = 8
B_PAD = CORE_ROWS * N_CORES   # 524288


def _bf16(a):
    import ml_dtypes
    return np.asarray(a, np.float32).astype(ml_dtypes.bfloat16)


def _const_layout():
    """Column offsets of each stationary inside the packed `big` tensor."""
    off = {}
    c = 0
    def add(name, w):
        nonlocal c
        off[name] = c
        c += w
    add("wx", 49)
    add("ident", 128)
    for q in range(1, NPAIR):
        for cc in range(q):
            add(f"a_{q}_{cc}", 128)
    for q in range(NPAIR):
        add(f"p_{q}", 64)
    for cc in range(NPAIR):
        add(f"wd_{cc}", 16)
    add("sd", 16)
    return off, c


def prep_consts(inputs):
    """Host-side packing of all stationaries for the pair-fused layout.

    Data layouts on device:
      cx tile  [128, NB]: row 64k + 12u + i  (k: t-block within pcx tile,
               i = 2q+h), row 64k+48 = x-part of the final logit diff.
      cxF      [128, NPAIR*NB]: partition 64h + 16u + t, block q of NB cols;
               holds scale*(x @ w_i) + bias for stage i = 2q + h.
      y2[cc]   [128, NB]: partition 64h + 16v + t = relu output of stage 2cc+h.
    """
    ws = [np.asarray(inputs[f"w{i}"], np.float32) for i in range(STACK)]
    gamma = np.asarray(inputs["gamma"], np.float32)
    beta = np.asarray(inputs["beta"], np.float32)
    mean = np.asarray(inputs["mean"], np.float32)
    var = np.asarray(inputs["var"], np.float32)
    wf = np.asarray(inputs["wf"], np.float32)
    bf = np.asarray(inputs["bf"], np.float32)

    s = gamma / np.sqrt(var + EPS)
    bsh = beta - mean * s
    wd = wf[:, 0] - wf[:, 1]
    bd = float(bf[0] - bf[1])

    off, width = _const_layout()
    big = np.zeros((128, width), np.float32)
    rng16 = np.arange(16)

    # x-projection weights: col 24h + 6u + q for stage i = 2q + h
    # (order chosen so the cx->cxF scatter balances to 3 AP dims);
    # col 48 = x part of logit diff
    o = off["wx"]
    for i in range(STACK):
        q, h = divmod(i, 2)
        for u in range(UNITS):
            big[:, o + 24 * h + 6 * u + q] = ws[i][4 * i:, u] * s[i, u]
    big[:, o + 48] = wd[4 * STACK:]

    big[:, off["ident"]:off["ident"] + 128] = np.eye(128)

    # chunk stationaries: src pair cc (stages 2cc+jj) -> dest pair q (stages 2q+h)
    for q in range(1, NPAIR):
        for cc in range(q):
            o = off[f"a_{q}_{cc}"]
            for jj in range(2):
                j = 2 * cc + jj
                for h in range(2):
                    i = 2 * q + h
                    for v in range(UNITS):
                        for u in range(UNITS):
                            val = ws[i][4 * (i - 1 - j) + v, u] * s[i, u]
                            big[64 * jj + 16 * v + rng16,
                                o + 64 * h + 16 * u + rng16] = val

    # intra-pair partials: y_{2q} -> stage 2q+1 (dest cols 16u+t, M=64)
    for q in range(NPAIR):
        i = 2 * q + 1
        o = off[f"p_{q}"]
        for v in range(UNITS):
            for u in range(UNITS):
                val = ws[i][v, u] * s[i, u]
                big[16 * v + rng16, o + 16 * u + rng16] = val

    # tail: wd coefficients, src pair cc -> single logit-diff row (cols = t)
    for cc in range(NPAIR):
        o = off[f"wd_{cc}"]
        for jj in range(2):
            j = 2 * cc + jj
            for v in range(UNITS):
                big[64 * jj + 16 * v + rng16, o + rng16] = wd[4 * (STACK - 1 - j) + v]

    # d-inject identity (cxFd row t -> pds row t)
    big[rng16, off["sd"] + rng16] = 1.0

    # per-partition bias folded into the pcx->cx copy: row 64k + 24h + 6u + q
    bias = np.zeros((128, 1), np.float32)
    for k in range(2):
        for u in range(UNITS):
            for i in range(STACK):
                q, h = divmod(i, 2)
                bias[64 * k + 24 * h + 6 * u + q, 0] = bsh[i, u]

    return {"big": _bf16(big), "wx": _bf16(big[:, off["wx"]:off["wx"] + 49]),
            "bias": bias, "bd": bd}


DEBUG = False
MM_LABELS = {}     # instruction name -> semantic label (for trace analysis)
_CUR = [""]


def build_kernel(ctx, tc, outs, ins, *, bd):
    import concourse.mybir as mybir

    nc = tc.nc
    f32 = mybir.dt.float32
    bf16 = mybir.dt.bfloat16
    ACT = mybir.ActivationFunctionType
    ALU = mybir.AluOpType

    xg_ap = ins["x"]          # [128, CORE_ROWS] bf16 (pre-transposed on host)
    out_ap = outs["out"]      # [CORE_ROWS, 2] f32

    _real_mm = nc.tensor.matmul

    def _mm(*a, **kw):
        inst = _real_mm(*a, **kw)
        try:
            MM_LABELS[inst.name] = _CUR[0]
        except Exception:
            pass
        return inst
    nc.tensor.matmul = _mm

    off, width = _const_layout()
    const_pool = ctx.enter_context(tc.tile_pool(name="consts", bufs=1))
    # tiny consts first so the x-projection can start immediately; the big
    # stationary pack loads on the gpsimd queue in the background
    wx_t = const_pool.tile([128, 49], bf16, tag="wx", name="wx_t")
    nc.sync.dma_start(wx_t[:], ins["wx"])
    bias_sb = const_pool.tile([128, 1], f32, tag="bias", name="bias_sb")
    nc.scalar.dma_start(bias_sb[:], ins["bias"])
    big = const_pool.tile([128, width], bf16, tag="big", name="big")
    nc.gpsimd.dma_start(big[:], ins["big"])

    wx = wx_t[:]
    ident = big[:, off["ident"]:off["ident"] + 128]
    a_st = {(q, cc): big[:, off[f"a_{q}_{cc}"]:off[f"a_{q}_{cc}"] + 128]
            for q in range(1, NPAIR) for cc in range(q)}
    p_st = {q: big[0:64, off[f"p_{q}"]:off[f"p_{q}"] + 64] for q in range(NPAIR)}
    wd_st = [big[:, off[f"wd_{cc}"]:off[f"wd_{cc}"] + 16] for cc in range(NPAIR)]
    sd = big[0:16, off["sd"]:off["sd"] + 16]

    xt_pool = ctx.enter_context(tc.tile_pool(name="xt", bufs=12))
    cx_pool = ctx.enter_context(tc.tile_pool(name="cx", bufs=6))
    cxf_pool = ctx.enter_context(tc.tile_pool(name="cxf", bufs=4))
    cxfd_pool = ctx.enter_context(tc.tile_pool(name="cxfd", bufs=4))
    y2_pool = ctx.enter_context(tc.tile_pool(name="y2", bufs=3))
    out_pool = ctx.enter_context(tc.tile_pool(name="outsb", bufs=2))
    # PSUM budget (8 banks): pcx ring 2 x 1 + z ring 2 x 2 + pds 2 x 1
    psum_pool = ctx.enter_context(tc.tile_pool(name="ps", bufs=2, space="PSUM"))
    pd_pool = ctx.enter_context(tc.tile_pool(name="pd", bufs=2, space="PSUM"))

    state = {}

    def start_group(g):
        cxF = cxf_pool.tile([128, NPAIR * NB], bf16, tag="cxF", name="cxF")
        cxFd = cxfd_pool.tile([16, NB], bf16, tag="cxFd", name="cxFd")
        y2 = [y2_pool.tile([128, NB], bf16, tag=f"y2c{cc}", name=f"y2c{cc}")
              for cc in range(NPAIR)]
        state[g] = (cxF, cxFd, y2)

    def relu_full(dst, src, flip):
        """Full-width relu on one act engine (fewer per-op overheads)."""
        if not flip:
            nc.scalar.activation(dst, src, ACT.Relu)
        else:
            nc.vector.tensor_scalar(dst, src, 0.0, None, ALU.max)

    def emit_cx_dma(g, j, eng=None):
        xt = xt_pool.tile([128, 2 * NB], bf16, tag="xt", name="xt")
        c0 = g * G_ROWS + j * 2 * NB
        (eng or nc.sync).dma_start(xt[:], xg_ap[:, c0:c0 + 2 * NB])
        return xt

    def emit_cx_compute(g, j, xt, pcx_tag="ps"):
        cxF, cxFd, y2 = state[g]
        _CUR[0] = f"cx{g}.{j}"
        # NB+64 pad: keeps the scatter's q-stride (one partition) from being
        # dim-merged with the 1024-element inner run into a bogus contiguous
        # 6144-element descriptor that would read past the partition.
        cxsb_full = cx_pool.tile([128, NB + 64], bf16, tag="cx")
        cxsb = cxsb_full[:, 0:NB]
        if pcx_tag == "z":
            # prologue-only: borrow a 2-bank z-ring slab for extra depth
            pcx = psum_pool.tile([128, NB], f32, tag="z", name="pcxz")
            for tb in range(2):
                for hh in range(2):
                    nc.tensor.matmul(
                        pcx[64 * tb:64 * tb + 49, 512 * hh:512 * hh + 512],
                        wx, xt[:, NB * tb + 512 * hh:NB * tb + 512 * hh + 512],
                        start=True, stop=True)
            if (j + g) % 2 == 0:
                nc.scalar.activation(cxsb[:], pcx[:], ACT.Identity,
                                     bias=bias_sb[:, 0:1])
            else:
                nc.vector.tensor_scalar(cxsb[:], pcx[:], bias_sb[:, 0:1],
                                        None, ALU.add)
        else:
            for hh in range(2):
                pcx = psum_pool.tile([128, 512], f32, tag="ps", name="pcx")
                for tb in range(2):
                    nc.tensor.matmul(
                        pcx[64 * tb:64 * tb + 49, :],
                        wx, xt[:, NB * tb + 512 * hh:NB * tb + 512 * hh + 512],
                        start=True, stop=True)
                sl = slice(512 * hh, 512 * hh + 512)
                if (j + g + hh) % 2 == 0:
                    nc.scalar.activation(cxsb[:, sl], pcx[:], ACT.Identity,
                                         bias=bias_sb[:, 0:1])
                else:
                    nc.vector.tensor_scalar(cxsb[:, sl], pcx[:], bias_sb[:, 0:1],
                                            None, ALU.add)
        # scatter rows 64k + 24h + 6u + q -> cxF[16*(4h+u) + t], block q.
        # src must stay 2D (the balancer splits 48 -> 8x6); an explicit 3D
        # src AP silently drops the middle dim in SW-DGE descriptor gen.
        for k in range(2):
            dst = cxF[:].rearrange(
                "(p s) (q n) -> p q s n", p=8, s=T, q=NPAIR)[:, :, 2 * j + k]
            eng = nc.sync if k == 0 else nc.gpsimd
            eng.dma_start(dst, cxsb[64 * k:64 * k + 48])
        dsrc = cxsb[:].rearrange("(k c) n -> k c n", k=2)[:, 48]
        nc.gpsimd.dma_start(cxFd[2 * j:2 * j + 2, :], dsrc)

    xt_pending = {}   # g -> list of prefetched xt tiles

    def prefetch_xt(g, upto, split=False):
        lst = xt_pending.setdefault(g, [])
        while len(lst) < min(upto, 8):
            eng = nc.scalar if (split and len(lst) % 2 == 1) else nc.sync
            lst.append(emit_cx_dma(g, len(lst), eng))

    def emit_pair(g, q, midfill=None, prefill=None):
        cxF, cxFd, y2 = state[g]
        z = psum_pool.tile([128, NB], f32, tag="z", name=f"z{q}", bufs=2)
        csl = slice(q * NB, (q + 1) * NB)
        _CUR[0] = f"id{g}.{q}"
        if q == 0:
            # stage 0 is pure relu(cx): read cxF directly, PSUM only for stage 1
            for hh in range(2):
                nc.tensor.matmul(z[64:128, 512 * hh:512 * hh + 512],
                                 ident[:, 64:128],
                                 cxF[:, q * NB + 512 * hh:q * NB + 512 * hh + 512],
                                 start=True, stop=False)
            relu_full(y2[0][0:64, :], cxF[0:64, csl], flip=g % 2 == 1)
        else:
            for hh in range(2):
                nc.tensor.matmul(z[:, 512 * hh:512 * hh + 512], ident,
                                 cxF[:, q * NB + 512 * hh:q * NB + 512 * hh + 512],
                                 start=True, stop=False)
            for cc in range(q):
                if cc == q - 1 and prefill is not None:
                    # filler covers the relu#2 latency of the previous pair
                    prefill()
                _CUR[0] = f"ch{g}.{q}.{cc}"
                for hh in range(2):
                    nc.tensor.matmul(z[:, 512 * hh:512 * hh + 512], a_st[(q, cc)],
                                     y2[cc][:, 512 * hh:512 * hh + 512],
                                     start=False, stop=False,
                                     skip_group_check=True)
            relu_full(y2[q][0:64, :], z[0:64, :], flip=(g + q) % 2 == 1)
        # filler matmuls here cover the relu#1 latency the partial waits on
        if midfill is not None:
            midfill()
        _CUR[0] = f"pa{g}.{q}"
        for hh in range(2):
            nc.tensor.matmul(z[64:128, 512 * hh:512 * hh + 512], p_st[q],
                             y2[q][0:64, 512 * hh:512 * hh + 512],
                             start=False, stop=True, skip_group_check=True)
        relu_full(y2[q][64:128, :], z[64:128, :], flip=(g + q) % 2 == 0)

    tail_state = {}

    def emit_tail_start(g):
        cxF, cxFd, y2 = state[g]
        _CUR[0] = f"tl{g}.s"
        pds = [pd_pool.tile([16, 512], f32, tag="pd", name=f"pds{hh}")
               for hh in range(2)]
        tail_state[g] = pds
        for hh in range(2):
            nc.tensor.matmul(pds[hh][:], sd, cxFd[:, 512 * hh:512 * hh + 512],
                             start=True, stop=False)

    def emit_tail_chunk(g, cc):
        cxF, cxFd, y2 = state[g]
        _CUR[0] = f"tl{g}.c{cc}"
        pds = tail_state[g]
        for hh in range(2):
            nc.tensor.matmul(pds[hh][:], wd_st[cc],
                             y2[cc][:, 512 * hh:512 * hh + 512],
                             start=False, stop=(cc == NPAIR - 1),
                             skip_group_check=True)

    def emit_tail_out(g):
        _CUR[0] = f"tl{g}.o"
        pds = tail_state[g]
        outsb = out_pool.tile([16, 2 * NB], f32, tag="outsb", name="outsb")
        for hh in range(2):
            o2 = outsb[:, 1024 * hh:1024 * hh + 1024].rearrange(
                "p (n two) -> p n two", two=2)
            nc.scalar.activation(o2[:, :, 0], pds[hh][:], ACT.Sigmoid,
                                 bias=float(bd))
            if g < GROUPS - 1:
                # p1 = 1 - p0 on gpsimd (SBUF-only, off the act engines); the
                # last group avoids gpsimd so its SW-DGE drain overlaps compute
                nc.gpsimd.tensor_scalar(o2[:, :, 1], o2[:, :, 0], -1.0, 1.0,
                                        ALU.mult, ALU.add)
            else:
                nc.scalar.activation(o2[:, :, 1], pds[hh][:], ACT.Sigmoid,
                                     bias=float(-bd), scale=-1.0)
        og = out_ap[g * G_ROWS:(g + 1) * G_ROWS, :].rearrange(
            "(t n) two -> t (n two)", t=T)
        nc.scalar.dma_start(og, outsb[:])

    def emit_debug(g):
        cxF, cxFd, y2 = state[g]
        nc.sync.dma_start(outs["dbg_cxf"], cxF[:])
        nc.sync.dma_start(outs["dbg_cxfd"], cxFd[:])
        for cc in range(NPAIR):
            nc.sync.dma_start(outs["dbg_y2"][:, cc * NB:(cc + 1) * NB], y2[cc][:])

    # Software pipeline: group g's pair recurrence interleaves group g+1's
    # x-projection tiles; the previous group's tail is woven in after pair 0.
    start_group(0)
    prefetch_xt(0, 8, split=True)
    prefetch_xt(1, 3, split=True)
    for j in range(8):
        emit_cx_compute(0, j, xt_pending[0][j])
    pending_tail = None
    for g in range(GROUPS):
        if g + 1 < GROUPS:
            start_group(g + 1)
            prefetch_xt(g + 1, 3)
        emitted = 0

        def fill(upto):
            nonlocal emitted
            if g + 1 >= GROUPS:
                return
            while emitted < min(upto, 8):
                emit_cx_compute(g + 1, emitted, xt_pending[g + 1][emitted])
                emitted += 1
                prefetch_xt(g + 1, emitted + 3)

        PRE = [0, 2, 4, 5, 7, 8]
        MID = [1, 3, 4, 6, 7, 8]

        def mid(q):
            if q == 0 and pending_tail is not None:
                emit_tail_chunk(pending_tail, 5)
                emit_tail_out(pending_tail)
            fill(MID[q])

        for q in range(NPAIR):
            emit_pair(g, q, midfill=lambda q=q: mid(q),
                      prefill=lambda q=q: fill(PRE[q]))
            if q == 0 and pending_tail is not None:
                pending_tail = None
            if q == 1:
                emit_tail_start(g)
            if q >= 1:
                emit_tail_chunk(g, q - 1)
        pending_tail = g
        if DEBUG and g == 0:
            emit_debug(0)
    emit_tail_chunk(pending_tail, 5)
    emit_tail_out(pending_tail)


# ---------------------------------------------------------------------------
# Self-contained entry point: kernel(**inputs) -> [500000, 2] float32
# ---------------------------------------------------------------------------

import sys as _sys
if '/opt/trn_rl_repo' not in _sys.path:
    _sys.path.insert(0, '/opt/trn_rl_repo')

_CACHE = {}


def _build_nc(bd):
    from contextlib import ExitStack
    import concourse.mybir as mybir
    from concourse import bacc
    import concourse.tile as tile

    _, width = _const_layout()
    nc = bacc.Bacc("TRN2", target_bir_lowering=False, debug=False,
                   num_devices=N_CORES)
    ins = {}
    ins["x"] = nc.dram_tensor("x", [128, CORE_ROWS], mybir.dt.bfloat16,
                              kind="ExternalInput").ap()
    ins["big"] = nc.dram_tensor("big", [128, width], mybir.dt.bfloat16,
                                kind="ExternalInput").ap()
    ins["wx"] = nc.dram_tensor("wx", [128, 49], mybir.dt.bfloat16,
                               kind="ExternalInput").ap()
    ins["bias"] = nc.dram_tensor("bias", [128, 1], mybir.dt.float32,
                                 kind="ExternalInput").ap()
    outs = {"out": nc.dram_tensor("out", [CORE_ROWS, 2], mybir.dt.float32,
                                  kind="ExternalOutput").ap()}
    if DEBUG:
        outs["dbg_cxf"] = nc.dram_tensor(
            "dbg_cxf", [128, NPAIR * NB], mybir.dt.bfloat16,
            kind="ExternalOutput").ap()
        outs["dbg_cxfd"] = nc.dram_tensor(
            "dbg_cxfd", [16, NB], mybir.dt.bfloat16, kind="ExternalOutput").ap()
        outs["dbg_y2"] = nc.dram_tensor(
            "dbg_y2", [128, NPAIR * NB], mybir.dt.bfloat16,
            kind="ExternalOutput").ap()
    with tile.TileContext(nc) as tc:
        with ExitStack() as ctx:
            build_kernel(ctx, tc, outs, ins, bd=bd)
    nc.compile()
    return nc


def make_in_maps(inputs):
    import ml_dtypes

    consts = prep_consts(inputs)
    bd = consts.pop("bd")
    x = np.asarray(inputs["x"], dtype=np.float32)
    assert x.shape == (B_FULL, D)
    xb = np.zeros((B_PAD, D), ml_dtypes.bfloat16)
    xb[:B_FULL] = x.astype(ml_dtypes.bfloat16)

    in_maps = []
    for c in range(N_CORES):
        xt = np.ascontiguousarray(xb[c * CORE_ROWS:(c + 1) * CORE_ROWS].T)
        m = {"x": xt}
        m.update(consts)
        in_maps.append(m)
    return in_maps, bd


def kernel(**inputs):
    from concourse.bass_utils import run_bass_kernel_spmd

    in_maps, bd = make_in_maps(inputs)
    if "nc" not in _CACHE:
        _CACHE["nc"] = _build_nc(bd)
    nc = _CACHE["nc"]
    res = run_bass_kernel_spmd(nc, in_maps, core_ids=list(range(N_CORES)))
    out = np.concatenate([res.results[c]["out"] for c in range(N_CORES)], axis=0)
    return out[:B_FULL]


# revision 55
# speedup vs baseline: 1.0513x; 1.0008x over previous
import numpy as np

STACK, UNITS, D, EPS = 12, 4, 128, 1e-3
T = 16                    # t-blocks per group (partition dim: 4 units x 16 t)
NB = 1024                 # cols per t-block
G_ROWS = T * NB           # 16384 rows per group
GROUPS = 4
NPAIR = STACK // 2        # 6 stage pairs
CORE_ROWS = GROUPS * G_ROWS   # 65536
B_FULL = 500000
N_CORES = 8
B_PAD = CORE_ROWS * N_CORES   # 524288


def _bf16(a):
    import ml_dtypes
    return np.asarray(a, np.float32).astype(ml_dtypes.bfloat16)


def _const_layout():
    """Column offsets of each stationary inside the packed `big` tensor."""
    off = {}
    c = 0
    def add(name, w):
        nonlocal c
        off[name] = c
        c += w
    add("wx", 49)
    add("ident", 128)
    for q in range(1, NPAIR):
        for cc in range(q):
            add(f"a_{q}_{cc}", 128)
    for q in range(NPAIR):
        add(f"p_{q}", 64)
    for cc in range(NPAIR):
        add(f"wd_{cc}", 16)
    add("sd", 16)
    return off, c


def prep_consts(inputs):
    """Host-side packing of all stationaries for the pair-fused layout.

    Data layouts on device:
      cx tile  [128, NB]: row 64k + 12u + i  (k: t-block within pcx tile,
               i = 2q+h), row 64k+48 = x-part of the final logit diff.
      cxF      [128, NPAIR*NB]: partition 64h + 16u + t, block q of NB cols;
               holds scale*(x @ w_i) + bias for stage i = 2q + h.
      y2[cc]   [128, NB]: partition 64h + 16v + t = relu output of stage 2cc+h.
    """
    ws = [np.asarray(inputs[f"w{i}"], np.float32) for i in range(STACK)]
    gamma = np.asarray(inputs["gamma"], np.float32)
    beta = np.asarray(inputs["beta"], np.float32)
    mean = np.asarray(inputs["mean"], np.float32)
    var = np.asarray(inputs["var"], np.float32)
    wf = np.asarray(inputs["wf"], np.float32)
    bf = np.asarray(inputs["bf"], np.float32)

    s = gamma / np.sqrt(var + EPS)
    bsh = beta - mean * s
    wd = wf[:, 0] - wf[:, 1]
    bd = float(bf[0] - bf[1])

    off, width = _const_layout()
    big = np.zeros((128, width), np.float32)
    rng16 = np.arange(16)

    # x-projection weights: col 24h + 6u + q for stage i = 2q + h
    # (order chosen so the cx->cxF scatter balances to 3 AP dims);
    # col 48 = x part of logit diff
    o = off["wx"]
    for i in range(STACK):
        q, h = divmod(i, 2)
        for u in range(UNITS):
            big[:, o + 24 * h + 6 * u + q] = ws[i][4 * i:, u] * s[i, u]
    big[:, o + 48] = wd[4 * STACK:]

    big[:, off["ident"]:off["ident"] + 128] = np.eye(128)

    # chunk stationaries: src pair cc (stages 2cc+jj) -> dest pair q (stages 2q+h)
    for q in range(1, NPAIR):
        for cc in range(q):
            o = off[f"a_{q}_{cc}"]
            for jj in range(2):
                j = 2 * cc + jj
                for h in range(2):
                    i = 2 * q + h
                    for v in range(UNITS):
                        for u in range(UNITS):
                            val = ws[i][4 * (i - 1 - j) + v, u] * s[i, u]
                            big[64 * jj + 16 * v + rng16,
                                o + 64 * h + 16 * u + rng16] = val

    # intra-pair partials: y_{2q} -> stage 2q+1 (dest cols 16u+t, M=64)
    for q in range(NPAIR):
        i = 2 * q + 1
        o = off[f"p_{q}"]
        for v in range(UNITS):
            for u in range(UNITS):
                val = ws[i][v, u] * s[i, u]
                big[16 * v + rng16, o + 16 * u + rng16] = val

    # tail: wd coefficients, src pair cc -> single logit-diff row (cols = t)
    for cc in range(NPAIR):
        o = off[f"wd_{cc}"]
        for jj in range(2):
            j = 2 * cc + jj
            for v in range(UNITS):
                big[64 * jj + 16 * v + rng16, o + rng16] = wd[4 * (STACK - 1 - j) + v]

    # d-inject identity (cxFd row t -> pds row t)
    big[rng16, off["sd"] + rng16] = 1.0

    # per-partition bias folded into the pcx->cx copy: row 64k + 24h + 6u + q
    bias = np.zeros((128, 1), np.float32)
    for k in range(2):
        for u in range(UNITS):
            for i in range(STACK):
                q, h = divmod(i, 2)
                bias[64 * k + 24 * h + 6 * u + q, 0] = bsh[i, u]

    return {"big": _bf16(big), "wx": _bf16(big[:, off["wx"]:off["wx"] + 49]),
            "bias": bias, "bd": bd}


DEBUG = False
MM_LABELS = {}     # instruction name -> semantic label (for trace analysis)
_CUR = [""]


def build_kernel(ctx, tc, outs, ins, *, bd):
    import concourse.mybir as mybir

    nc = tc.nc
    f32 = mybir.dt.float32
    bf16 = mybir.dt.bfloat16
    ACT = mybir.ActivationFunctionType
    ALU = mybir.AluOpType

    xg_ap = ins["x"]          # [128, CORE_ROWS] bf16 (pre-transposed on host)
    out_ap = outs["out"]      # [CORE_ROWS, 2] f32

    _real_mm = nc.tensor.matmul

    def _mm(*a, **kw):
        inst = _real_mm(*a, **kw)
        try:
            MM_LABELS[inst.name] = _CUR[0]
        except Exception:
            pass
        return inst
    nc.tensor.matmul = _mm

    off, width = _const_layout()
    const_pool = ctx.enter_context(tc.tile_pool(name="consts", bufs=1))
    # tiny consts first so the x-projection can start immediately; the big
    # stationary pack loads on the gpsimd queue in the background
    wx_t = const_pool.tile([128, 49], bf16, tag="wx", name="wx_t")
    nc.sync.dma_start(wx_t[:], ins["wx"])
    bias_sb = const_pool.tile([128, 1], f32, tag="bias", name="bias_sb")
    nc.scalar.dma_start(bias_sb[:], ins["bias"])
    big = const_pool.tile([128, width], bf16, tag="big", name="big")
    nc.gpsimd.dma_start(big[:], ins["big"])

    wx = wx_t[:]
    ident = big[:, off["ident"]:off["ident"] + 128]
    a_st = {(q, cc): big[:, off[f"a_{q}_{cc}"]:off[f"a_{q}_{cc}"] + 128]
            for q in range(1, NPAIR) for cc in range(q)}
    p_st = {q: big[0:64, off[f"p_{q}"]:off[f"p_{q}"] + 64] for q in range(NPAIR)}
    wd_st = [big[:, off[f"wd_{cc}"]:off[f"wd_{cc}"] + 16] for cc in range(NPAIR)]
    sd = big[0:16, off["sd"]:off["sd"] + 16]

    xt_pool = ctx.enter_context(tc.tile_pool(name="xt", bufs=12))
    cx_pool = ctx.enter_context(tc.tile_pool(name="cx", bufs=6))
    cxf_pool = ctx.enter_context(tc.tile_pool(name="cxf", bufs=4))
    cxfd_pool = ctx.enter_context(tc.tile_pool(name="cxfd", bufs=4))
    y2_pool = ctx.enter_context(tc.tile_pool(name="y2", bufs=3))
    out_pool = ctx.enter_context(tc.tile_pool(name="outsb", bufs=2))
    # PSUM budget (8 banks): pcx ring 2 x 1 + z ring 2 x 2 + pds 2 x 1
    psum_pool = ctx.enter_context(tc.tile_pool(name="ps", bufs=2, space="PSUM"))
    pd_pool = ctx.enter_context(tc.tile_pool(name="pd", bufs=2, space="PSUM"))

    state = {}

    def start_group(g):
        cxF = cxf_pool.tile([128, NPAIR * NB], bf16, tag="cxF", name="cxF")
        cxFd = cxfd_pool.tile([16, NB], bf16, tag="cxFd", name="cxFd")
        y2 = [y2_pool.tile([128, NB], bf16, tag=f"y2c{cc}", name=f"y2c{cc}")
              for cc in range(NPAIR)]
        state[g] = (cxF, cxFd, y2)

    def relu_full(dst, src, flip):
        """Full-width relu on one act engine (fewer per-op overheads)."""
        if not flip:
            nc.scalar.activation(dst, src, ACT.Relu)
        else:
            nc.vector.tensor_scalar(dst, src, 0.0, None, ALU.max)

    def emit_cx_dma(g, j, eng=None):
        xt = xt_pool.tile([128, 2 * NB], bf16, tag="xt", name="xt")
        c0 = g * G_ROWS + j * 2 * NB
        (eng or nc.sync).dma_start(xt[:], xg_ap[:, c0:c0 + 2 * NB])
        return xt

    def emit_cx_compute(g, j, xt, pcx_tag="ps"):
        cxF, cxFd, y2 = state[g]
        _CUR[0] = f"cx{g}.{j}"
        # NB+64 pad: keeps the scatter's q-stride (one partition) from being
        # dim-merged with the 1024-element inner run into a bogus contiguous
        # 6144-element descriptor that would read past the partition.
        cxsb_full = cx_pool.tile([128, NB + 64], bf16, tag="cx")
        cxsb = cxsb_full[:, 0:NB]
        if pcx_tag == "z":
            # prologue-only: borrow a 2-bank z-ring slab for extra depth
            pcx = psum_pool.tile([128, NB], f32, tag="z", name="pcxz")
            for tb in range(2):
                for hh in range(2):
                    nc.tensor.matmul(
                        pcx[64 * tb:64 * tb + 49, 512 * hh:512 * hh + 512],
                        wx, xt[:, NB * tb + 512 * hh:NB * tb + 512 * hh + 512],
                        start=True, stop=True)
            if (j + g) % 2 == 0:
                nc.scalar.activation(cxsb[:], pcx[:], ACT.Identity,
                                     bias=bias_sb[:, 0:1])
            else:
                nc.vector.tensor_scalar(cxsb[:], pcx[:], bias_sb[:, 0:1],
                                        None, ALU.add)
        else:
            for hh in range(2):
                pcx = psum_pool.tile([128, 512], f32, tag="ps", name="pcx")
                for tb in range(2):
                    nc.tensor.matmul(
                        pcx[64 * tb:64 * tb + 49, :],
                        wx, xt[:, NB * tb + 512 * hh:NB * tb + 512 * hh + 512],
                        start=True, stop=True)
                sl = slice(512 * hh, 512 * hh + 512)
                if (j + g + hh) % 2 == 0:
                    nc.scalar.activation(cxsb[:, sl], pcx[:], ACT.Identity,
                                         bias=bias_sb[:, 0:1])
                else:
                    nc.vector.tensor_scalar(cxsb[:, sl], pcx[:], bias_sb[:, 0:1],
                                            None, ALU.add)
        # scatter rows 64k + 24h + 6u + q -> cxF[16*(4h+u) + t], block q.
        # src must stay 2D (the balancer splits 48 -> 8x6); an explicit 3D
        # src AP silently drops the middle dim in SW-DGE descriptor gen.
        for k in range(2):
            dst = cxF[:].rearrange(
                "(p s) (q n) -> p q s n", p=8, s=T, q=NPAIR)[:, :, 2 * j + k]
            eng = nc.sync if k == 0 else nc.gpsimd
            eng.dma_start(dst, cxsb[64 * k:64 * k + 48])
        dsrc = cxsb[:].rearrange("(k c) n -> k c n", k=2)[:, 48]
        nc.gpsimd.dma_start(cxFd[2 * j:2 * j + 2, :], dsrc)

    xt_pending = {}   # g -> list of prefetched xt tiles

    def prefetch_xt(g, upto, split=False):
        lst = xt_pending.setdefault(g, [])
        while len(lst) < min(upto, 8):
            eng = nc.scalar if (split and len(lst) % 2 == 1) else nc.sync
            lst.append(emit_cx_dma(g, len(lst), eng))

    def emit_pair(g, q, midfill=None, prefill=None):
        cxF, cxFd, y2 = state[g]
        z = psum_pool.tile([128, NB], f32, tag="z", name=f"z{q}", bufs=2)
        csl = slice(q * NB, (q + 1) * NB)
        _CUR[0] = f"id{g}.{q}"
        if q == 0:
            # stage 0 is pure relu(cx): read cxF directly, PSUM only for stage 1
            for hh in range(2):
                nc.tensor.matmul(z[64:128, 512 * hh:512 * hh + 512],
                                 ident[:, 64:128],
                                 cxF[:, q * NB + 512 * hh:q * NB + 512 * hh + 512],
                                 start=True, stop=False)
            relu_full(y2[0][0:64, :], cxF[0:64, csl], flip=g % 2 == 1)
        else:
            for hh in range(2):
                nc.tensor.matmul(z[:, 512 * hh:512 * hh + 512], ident,
                                 cxF[:, q * NB + 512 * hh:q * NB + 512 * hh + 512],
                                 start=True, stop=False)
            for cc in range(q):
                if cc == q - 1 and prefill is not None:
                    # filler covers the relu#2 latency of the previous pair
                    prefill()
                _CUR[0] = f"ch{g}.{q}.{cc}"
                for hh in range(2):
                    nc.tensor.matmul(z[:, 512 * hh:512 * hh + 512], a_st[(q, cc)],
                                     y2[cc][:, 512 * hh:512 * hh + 512],
                                     start=False, stop=False,
                                     skip_group_check=True)
            relu_full(y2[q][0:64, :], z[0:64, :], flip=(g + q) % 2 == 1)
        # filler matmuls here cover the relu#1 latency the partial waits on
        if midfill is not None:
            midfill()
        _CUR[0] = f"pa{g}.{q}"
        for hh in range(2):
            nc.tensor.matmul(z[64:128, 512 * hh:512 * hh + 512], p_st[q],
                             y2[q][0:64, 512 * hh:512 * hh + 512],
                             start=False, stop=True, skip_group_check=True)
        relu_full(y2[q][64:128, :], z[64:128, :], flip=(g + q) % 2 == 0)

    tail_state = {}

    def emit_tail_start(g):
        cxF, cxFd, y2 = state[g]
        _CUR[0] = f"tl{g}.s"
        pds = [pd_pool.tile([16, 512], f32, tag="pd", name=f"pds{hh}")
               for hh in range(2)]
        tail_state[g] = pds
        for hh in range(2):
            nc.tensor.matmul(pds[hh][:], sd, cxFd[:, 512 * hh:512 * hh + 512],
                             start=True, stop=False)

    def emit_tail_chunk(g, cc):
        cxF, cxFd, y2 = state[g]
        _CUR[0] = f"tl{g}.c{cc}"
        pds = tail_state[g]
        for hh in range(2):
            nc.tensor.matmul(pds[hh][:], wd_st[cc],
                             y2[cc][:, 512 * hh:512 * hh + 512],
                             start=False, stop=(cc == NPAIR - 1),
                             skip_group_check=True)

    def emit_tail_out(g):
        _CUR[0] = f"tl{g}.o"
        pds = tail_state[g]
        outsb = out_pool.tile([16, 2 * NB], f32, tag="outsb", name="outsb")
        for hh in range(2):
            o2 = outsb[:, 1024 * hh:1024 * hh + 1024].rearrange(
                "p (n two) -> p n two", two=2)
            nc.scalar.activation(o2[:, :, 0], pds[hh][:], ACT.Sigmoid,
                                 bias=float(bd))
            if g < GROUPS - 1:
                # p1 = 1 - p0 on gpsimd (SBUF-only, off the act engines); the
                # last group avoids gpsimd so its SW-DGE drain overlaps compute
                nc.gpsimd.tensor_scalar(o2[:, :, 1], o2[:, :, 0], -1.0, 1.0,
                                        ALU.mult, ALU.add)
            else:
                nc.scalar.activation(o2[:, :, 1], pds[hh][:], ACT.Sigmoid,
                                     bias=float(-bd), scale=-1.0)
        og = out_ap[g * G_ROWS:(g + 1) * G_ROWS, :].rearrange(
            "(t n) two -> t (n two)", t=T)
        nc.scalar.dma_start(og, outsb[:])

    def emit_debug(g):
        cxF, cxFd, y2 = state[g]
        nc.sync.dma_start(outs["dbg_cxf"], cxF[:])
        nc.sync.dma_start(outs["dbg_cxfd"], cxFd[:])
        for cc in range(NPAIR):
            nc.sync.dma_start(outs["dbg_y2"][:, cc * NB:(cc + 1) * NB], y2[cc][:])

    # Software pipeline: group g's pair recurrence interleaves group g+1's
    # x-projection tiles; the previous group's tail is woven in after pair 0.
    start_group(0)
    prefetch_xt(0, 8, split=True)
    prefetch_xt(1, 3, split=True)
    for j in range(8):
        emit_cx_compute(0, j, xt_pending[0][j], pcx_tag=("z" if j % 2 else "ps"))
    pending_tail = None
    for g in range(GROUPS):
        if g + 1 < GROUPS:
            start_group(g + 1)
            prefetch_xt(g + 1, 3)
        emitted = 0

        def fill(upto):
            nonlocal emitted
            if g + 1 >= GROUPS:
                return
            while emitted < min(upto, 8):
                emit_cx_compute(g + 1, emitted, xt_pending[g + 1][emitted])
                emitted += 1
                prefetch_xt(g + 1, emitted + 3)

        PRE = [0, 2, 4, 5, 7, 8]
        MID = [1, 3, 4, 6, 7, 8]

        def mid(q):
            if q == 0 and pending_tail is not None:
                emit_tail_chunk(pending_tail, 5)
                emit_tail_out(pending_tail)
            fill(MID[q])

        for q in range(NPAIR):
            emit_pair(g, q, midfill=lambda q=q: mid(q),
                      prefill=lambda q=q: fill(PRE[q]))
            if q == 0 and pending_tail is not None:
                pending_tail = None
            if q == 1:
                emit_tail_start(g)
            if q >= 1:
                emit_tail_chunk(g, q - 1)
        pending_tail = g
        if DEBUG and g == 0:
            emit_debug(0)
    emit_tail_chunk(pending_tail, 5)
    emit_tail_out(pending_tail)


# ---------------------------------------------------------------------------
# Self-contained entry point: kernel(**inputs) -> [500000, 2] float32
# ---------------------------------------------------------------------------

import sys as _sys
if '/opt/trn_rl_repo' not in _sys.path:
    _sys.path.insert(0, '/opt/trn_rl_repo')

_CACHE = {}


def _build_nc(bd):
    from contextlib import ExitStack
    import concourse.mybir as mybir
    from concourse import bacc
    import concourse.tile as tile

    _, width = _const_layout()
    nc = bacc.Bacc("TRN2", target_bir_lowering=False, debug=False,
                   num_devices=N_CORES)
    ins = {}
    ins["x"] = nc.dram_tensor("x", [128, CORE_ROWS], mybir.dt.bfloat16,
                              kind="ExternalInput").ap()
    ins["big"] = nc.dram_tensor("big", [128, width], mybir.dt.bfloat16,
                                kind="ExternalInput").ap()
    ins["wx"] = nc.dram_tensor("wx", [128, 49], mybir.dt.bfloat16,
                               kind="ExternalInput").ap()
    ins["bias"] = nc.dram_tensor("bias", [128, 1], mybir.dt.float32,
                                 kind="ExternalInput").ap()
    outs = {"out": nc.dram_tensor("out", [CORE_ROWS, 2], mybir.dt.float32,
                                  kind="ExternalOutput").ap()}
    if DEBUG:
        outs["dbg_cxf"] = nc.dram_tensor(
            "dbg_cxf", [128, NPAIR * NB], mybir.dt.bfloat16,
            kind="ExternalOutput").ap()
        outs["dbg_cxfd"] = nc.dram_tensor(
            "dbg_cxfd", [16, NB], mybir.dt.bfloat16, kind="ExternalOutput").ap()
        outs["dbg_y2"] = nc.dram_tensor(
            "dbg_y2", [128, NPAIR * NB], mybir.dt.bfloat16,
            kind="ExternalOutput").ap()
    with tile.TileContext(nc) as tc:
        with ExitStack() as ctx:
            build_kernel(ctx, tc, outs, ins, bd=bd)
    nc.compile()
    return nc


def make_in_maps(inputs):
    import ml_dtypes

    consts = prep_consts(inputs)
    bd = consts.pop("bd")
    x = np.asarray(inputs["x"], dtype=np.float32)
    assert x.shape == (B_FULL, D)
    xb = np.zeros((B_PAD, D), ml_dtypes.bfloat16)
    xb[:B_FULL] = x.astype(ml_dtypes.bfloat16)

    in_maps = []
    for c in range(N_CORES):
        xt = np.ascontiguousarray(xb[c * CORE_ROWS:(c + 1) * CORE_ROWS].T)
        m = {"x": xt}
        m.update(consts)
        in_maps.append(m)
    return in_maps, bd


def kernel(**inputs):
    from concourse.bass_utils import run_bass_kernel_spmd

    in_maps, bd = make_in_maps(inputs)
    if "nc" not in _CACHE:
        _CACHE["nc"] = _build_nc(bd)
    nc = _CACHE["nc"]
    res = run_bass_kernel_spmd(nc, in_maps, core_ids=list(range(N_CORES)))
    out = np.concatenate([res.results[c]["out"] for c in range(N_CORES)], axis=0)
    return out[:B_FULL]
